# revision 26
# baseline (speedup 1.0000x reference)
"""Trainium2 Bass kernel for nn_Encoder segment-reduce.

Reference computation (per sample b):
    cls = onehot(argmax_k outputs[b])            # [K, HW]
    sizes = cls.sum(HW) + 0.01                   # [K]
    feat_set = feats[b] @ cls.T / sizes          # [F, K]
    out[b] = w_proj @ feat_set + bias            # [E, K]

Kernel strategy (pure data parallel: 1 sample per NeuronCore, 8 cores).

Segment-reduce FIRST (the cheap contraction), projection second:
    feat_setT[k, f] = sum_hw onehot[hw, k] * featsT[hw, f]
with the onehot chunk [128hw, 21] as the PE's stationary operand and featsT
chunks [128hw, 512f] as the moving operand.  The four f-group matmuls of each
hw chunk are packed into the four 32-column groups of the PE array via
tile_position=(0, 32j): the stationary onehot only occupies 21 of 128 array
columns, so the four matmuls execute concurrently (measured 4ns stagger) and
the stream keeps pace with the DMA.  One [128, 512] PSUM tile holds all four
accumulators (f-group j at partitions 32j..32j+21).

feats dtype is fp8 e3m4 (TRN FP8_EXP3): N(0,1) data fits the e3m4 range and
its 4 mantissa bits give rel err ~1.2e-2 end to end (threshold 2e-2), halving
HBM traffic vs bf16: 8.39 MB feats + 1 MB wT (bf16) + 0.34 MB outputs (f32)
~= 9.75 MB/core -- the kernel is DMA-bound at the per-core HBM limit.  The
matmuls run in normal (single-rate) fp8 mode: double-fp8 would upcast
operands to e6m3 and destroy e3m4's 4th mantissa bit (measured 3.0e-2).

All host-side layouts give every DMA >= 2KB contiguous per-partition runs
(wT is pre-permuted to [p, fc, e]; bias rides in the outputs transfer).
DMA order: outputs+bias first (the DVE argmax fills the initial feats
window), feats in 1-2MB blocks, the first quarter of wT (needed by the first
projection round) just before the last feats block, then the rest of wT.
The per-chunk size-count matmul is interleaved with the stream so the PE
never serializes behind the (DVE-paced) argmax; the warm-up burst is sized
to end just as the first feats block lands (a longer burst delays the
stream: the PE executes in order at the cold 1.2 GHz clock).

Tail (all PE stages packed into array tile groups, dummy-matmul fillers in
the dependency gaps so the HAM clock gate keeps the PE at 2.4 GHz):
  - 1/sizes is replicated to all four column groups with one bf16 matmul,
    then fused into the PSUM->SBUF copies (DVE + ACT halves);
  - the 16 [21,128]->[128,21] transposes run 4-at-a-time (one per 32-row
    row group, via a replicated identity) into ONE [128, 16*21] PSUM tile,
    drained by two half-copies (DVE + ACT);
  - the 16 projection matmuls (feat_set chunk stationary, wT moving) run
    4-at-a-time in the four column groups (round r takes f-chunks 4r..4r+3),
    giving four partial [21, 256] sums at partitions 32j; one final matmul
    against the replicated identity combines them, with the bias pre-loaded
    into its PSUM accumulator early via a diagonal-bias matmul.
The output is stored as [K, E]; the host transposes when gathering.

dtype: "fp8" (e3m4 feats, rel err ~1.2e-2) or "bf16" (rel err ~3e-3).
"""

import numpy as np

import concourse.bacc as bacc
import concourse.bass as bass
import concourse.mybir as mybir
import concourse.tile as tile
from concourse.bass import ds, ts
from concourse.bass_utils import run_bass_kernel_spmd
from concourse.masks import make_identity

# Problem shapes (hardcoded per contract)
B = 8
K = 21
H = 64
W = 64
HW = H * W            # 4096
F = 2048
E = 256
P = 128
FC = F // P           # 16 f-chunks of 128
FG = 4                # f-groups of 512 (PE column groups)
FGW = F // FG         # 512
N_T = HW // P         # 32 hw chunks
OUT_AUG = N_T * K + 2  # outputs row + 2 bias values per partition
N_CORES = 8

F32 = mybir.dt.float32
BF16 = mybir.dt.bfloat16
FP8 = mybir.dt.float8e3   # e3m4: 4 mantissa bits

DTYPE = "fp8"         # "fp8" or "bf16"


def build_module(dtype=DTYPE, warmup=120, endwarm=30):
    mm_dt = FP8 if dtype == "fp8" else BF16
    nc = bacc.Bacc("TRN2", target_bir_lowering=False, debug=False)

    # outputs host-transposed to [p, t, k] (pixel-major).
    outputs_d = nc.dram_tensor("outputs_in", [P, N_T, K], F32, kind="ExternalInput")
    # featsT host-permuted to [p, t, fgrp, fj]: featsT[t*128+p, fgrp*512+fj].
    feats_d = nc.dram_tensor(
        "feats_in", [P, N_T, FG, FGW], mm_dt, kind="ExternalInput"
    )
    # wT host-permuted to [p, fc, e] = w_proj.T[fc*128+p, e] (contiguous 8KB
    # per-partition runs -- the naive [F, E] layout DMAs in 512B pieces).
    wT_d = nc.dram_tensor("wT_in", [P, FC, E], BF16, kind="ExternalInput")
    bias_d = nc.dram_tensor("bias_in", [E], F32, kind="ExternalInput")
    # out.T -- the host transposes each sample's [K, E] result when gathering.
    out_d = nc.dram_tensor("out", [K, E], F32, kind="ExternalOutput")

    # feats DMA blocks (start chunk, n chunks).
    blocks_a = [(0, 4), (4, 8), (12, 8), (20, 8)]
    blocks_b = [(28, 4)]

    with tile.TileContext(nc) as tc:
        with (
            tc.tile_pool(name="consts", bufs=1) as consts,
            tc.tile_pool(name="feats", bufs=5) as feats_pool,
            tc.tile_pool(name="small", bufs=4) as small,
            tc.tile_pool(name="outp", bufs=1) as outp,
            tc.tile_pool(name="ps_fs", bufs=1, space="PSUM") as ps_fs,
            tc.tile_pool(name="ps_sz", bufs=1, space="PSUM") as ps_sz,
            tc.tile_pool(name="ps_tr", bufs=1, space="PSUM") as ps_tr,
            tc.tile_pool(name="ps_misc", bufs=1, space="PSUM") as ps_misc,
        ):
            # Tiny warm-up matmuls: 16 columns x 32 rows (~3% of the PE
            # array) register as PE activity for the HAM clock governor --
            # lifting the clock to 2.4 GHz -- while consuming almost none of
            # its utilization budget (full-size dummy matmuls earn a
            # half-clock throttle debt that lands on the real work).
            warm_w = consts.tile([32, 16], BF16)
            warm_ps = ps_misc.tile([16, 16], F32, tag="warm")

            def warm(n):
                for _ in range(n):
                    nc.tensor.matmul(warm_ps, lhsT=warm_w, rhs=warm_w)

            nc.vector.memset(warm_w, 0.0)

            # Bulk DMAs in FIFO order on the sync HWDGE queue.
            outputs_sb = consts.tile([P, N_T, K], F32)
            nc.sync.dma_start(out=outputs_sb, in_=outputs_d.ap())
            bias_sb = consts.tile([1, E], F32)
            nc.sync.dma_start(
                out=bias_sb, in_=bias_d.ap().rearrange("(o e) -> o e", o=1)
            )
            feats_r = feats_d.ap()
            wT_sb = consts.tile([P, FC, E], BF16)
            fgs = {}

            def load_feats(block_list):
                for t0, tb in block_list:
                    fg = feats_pool.tile([P, tb, FG, FGW], mm_dt,
                                         name=f"fg{t0}", tag="fg")
                    nc.sync.dma_start(out=fg, in_=feats_r[:, ds(t0, tb)])
                    fgs[t0] = fg

            load_feats(blocks_a)
            load_feats(blocks_b)
            nc.sync.dma_start(out=wT_sb, in_=wT_d.ap())
            blocks = blocks_a + blocks_b
            warm(warmup)

            # Constants.  rep_sb[k, 32j+k'] = delta(k,k') replicates a [21]
            # partition vector to all four 32-row column-group offsets; it is
            # built with free-dim-shifted copies of the identity (partition
            # shifts are impossible on DVE, free shifts are not).
            ident = consts.tile([P, P], F32)
            make_identity(nc, ident)
            rep_sb = consts.tile([K, P], BF16)
            nc.vector.memset(rep_sb, 0.0)
            for j in range(FG):
                nc.vector.tensor_copy(rep_sb[:, ds(32 * j, K)], ident[:K, :K])
            ident21_b = consts.tile([K, K], BF16)
            nc.vector.tensor_copy(ident21_b, ident[:K, :K])
            # ident_rep[32j+k, k'] = delta(k,k'): per-column-group identity
            # for the tail transposes and the partial-sum combine, built on
            # the PE (rep_sb.T @ I21).
            identrep_ps = ps_tr.tile([P, K], F32, tag="t0")
            nc.tensor.matmul(
                identrep_ps, lhsT=rep_sb, rhs=ident21_b,
                start=True, stop=True,
            )
            ident_rep = consts.tile([P, K], BF16)
            nc.vector.tensor_copy(ident_rep, identrep_ps)
            ones_b = consts.tile([P, 2], mm_dt)
            nc.vector.memset(ones_b, 1.0)
            one1_b = consts.tile([1, 1], BF16)
            nc.vector.memset(one1_b, 1.0)
            ones128_21 = consts.tile([P, K], BF16)
            nc.vector.memset(ones128_21, 1.0)
            ones21r = consts.tile([1, K], BF16)
            nc.vector.memset(ones21r, 1.0)
            bias_b = consts.tile([1, E], BF16)
            nc.vector.tensor_copy(bias_b, bias_sb)
            # Force the ACT engine's table load off the critical path.
            dummy_act = small.tile([1, 2], F32, tag="da")
            nc.scalar.activation(
                out=dummy_act, in_=bias_sb[:, 0:2],
                func=mybir.ActivationFunctionType.Copy,
            )

            # Phase 1 (DVE only): onehot = (outT == rowmax) per hw chunk.
            oh_all = consts.tile([P, N_T, K], mm_dt)
            for t in range(N_T):
                rowmax = small.tile([P, 1], F32)
                nc.vector.tensor_reduce(
                    rowmax, outputs_sb[:, t, :], mybir.AxisListType.X,
                    mybir.AluOpType.max,
                )
                nc.vector.tensor_scalar(
                    out=oh_all[:, t, :],
                    in0=outputs_sb[:, t, :],
                    scalar1=rowmax,
                    scalar2=None,
                    op0=mybir.AluOpType.is_equal,
                )

            # Segment-reduce stream.  Per hw chunk: one size-count matmul
            # (onehot.T @ ones -> [21, 2]) plus four f-group matmuls packed
            # into the four PE column groups, accumulating [128, 512] PSUM
            # (f-group j at partitions 32j..32j+21) across all 32 chunks.
            fs_ps = ps_fs.tile([P, FGW], F32)
            szT_ps = ps_sz.tile([2, K], F32, tag="sz")
            for t0, tb in blocks:
                fg = fgs[t0]
                for ti in range(tb):
                    t = t0 + ti
                    oh_t = oh_all[:, t, :]
                    nc.tensor.matmul(
                        szT_ps,
                        lhsT=ones_b,
                        rhs=oh_t,
                        start=(t == 0),
                        stop=(t == N_T - 1),
                    )
                    for j in range(FG):
                        nc.tensor.matmul(
                            fs_ps[ds(32 * j, K), :],
                            lhsT=oh_t,
                            rhs=fg[:, ti, j, :],
                            start=(t == 0),
                            stop=(t == N_T - 1),
                            tile_position=(0, 32 * j),
                        )

            warm(endwarm)

            # 1/sizes: computed on the [1, 21] row, transposed to a [21, 1]
            # column with a tiny matmul, then replicated to all four column
            # groups on the PE.  The PE sits idle through this window, which
            # also pays down the HAM utilization budget before the tail.
            sizesT = small.tile([1, K], F32, tag="sizesT")
            nc.vector.tensor_scalar_add(sizesT, szT_ps[0:1, :], 0.01)
            recipT = small.tile([1, K], F32, tag="recipT")
            nc.vector.reciprocal(recipT, sizesT)
            recipT_b = small.tile([1, K], BF16, tag="recipTb")
            nc.vector.tensor_copy(recipT_b, recipT)
            recipc_ps = ps_sz.tile([K, 1], F32, name="recipc", tag="sz")
            nc.tensor.matmul(recipc_ps, lhsT=recipT_b, rhs=one1_b,
                             start=True, stop=True)
            recip_b = small.tile([K, 1], BF16, tag="recipb")
            nc.vector.tensor_copy(recip_b, recipc_ps)
            recip_ps = ps_sz.tile([P, 1], F32, name="recip128", tag="sz")
            nc.tensor.matmul(recip_ps, lhsT=rep_sb, rhs=recip_b,
                             start=True, stop=True)
            recip128 = small.tile([P, 1], F32, tag="r128")
            nc.vector.tensor_copy(recip128, recip_ps)

            # Scale by 1/sizes during the PSUM->SBUF copy (DVE + ACT halves).
            fs_sc = consts.tile([P, FGW], BF16)
            nc.vector.tensor_scalar_mul(
                fs_sc[:, 0 : FGW // 2], fs_ps[:, 0 : FGW // 2], recip128
            )
            nc.scalar.activation(
                out=fs_sc[:, ds(FGW // 2, FGW // 2)],
                in_=fs_ps[:, ds(FGW // 2, FGW // 2)],
                func=mybir.ActivationFunctionType.Copy,
                scale=recip128,
            )

            # Transposes, 4 concurrent per round (one per 32-row row group),
            # all into one PSUM tile, drained by two half-copies.  f-chunk
            # fc = 4j + c lives in column group j at free cols 128c..128c+128
            # of fs_sc.
            # (stride padded to 22 elements so each bf16 PSUM write is
            # 4-byte aligned)
            fsT_sb = consts.tile([P, FC, K], BF16)
            for c in range(4):
                trps = []
                for j in range(FG):
                    fc = 4 * j + c
                    trp = ps_tr.tile([P, K], BF16, name=f"trp{fc}",
                                     tag=f"t{j}")
                    nc.tensor.transpose(
                        trp,
                        fs_sc[ds(32 * j, K), ts(c, P)],
                        ident_rep[ds(32 * j, K), :],
                        tile_position=(32 * j, 0),
                    )
                    trps.append((fc, trp))
                for i, (fc, trp) in enumerate(trps):
                    if i % 2 == 0:
                        nc.vector.tensor_copy(fsT_sb[:, fc, :], trp)
                    else:
                        nc.scalar.activation(
                            out=fsT_sb[:, fc, :], in_=trp,
                            func=mybir.ActivationFunctionType.Copy,
                        )

            # Projection, 4 concurrent per round: round r covers f-chunks
            # 4r..4r+3; column group j accumulates partials at partitions 32j.
            proj_ps = ps_misc.tile([P, E], F32, tag="warm", name="proj")
            for r in range(4):
                for j in range(FG):
                    fc = 4 * r + j
                    nc.tensor.matmul(
                        proj_ps[ds(32 * j, K), :],
                        lhsT=fsT_sb[:, fc, :],
                        rhs=wT_sb[:, fc, :],
                        start=(r == 0),
                        stop=(r == 3),
                        tile_position=(0, 32 * j),
                    )
            proj_sb = consts.tile([P, E], BF16)
            nc.vector.tensor_copy(proj_sb, proj_ps)

            # Combine the four partials with the bias:
            # final[k, e] = bias[e] + sum_j proj[32j+k, e].
            final_ps = ps_misc.tile([K, E], F32, tag="warm", name="final")
            nc.tensor.matmul(final_ps, lhsT=ones21r, rhs=bias_b,
                             start=True, stop=False, skip_group_check=True)
            nc.tensor.matmul(final_ps, lhsT=ident_rep, rhs=proj_sb,
                             start=False, stop=True, skip_group_check=True)
            out_sb = outp.tile([K, E], F32)
            nc.vector.tensor_copy(out_sb, final_ps)
            nc.sync.dma_start(out=out_d.ap(), in_=out_sb)

    nc.compile()
    return nc


_CACHE = {}


def make_in_maps(outputs, feats, w_proj, b_proj, dtype=DTYPE):
    import ml_dtypes

    mm_np = ml_dtypes.float8_e3m4 if dtype == "fp8" else ml_dtypes.bfloat16
    outputs = np.asarray(outputs, dtype=np.float32)
    outputs_aug = np.ascontiguousarray(
        outputs.reshape(B, K, N_T, P).transpose(0, 3, 2, 1)
    )
    bias = np.ascontiguousarray(np.asarray(b_proj, dtype=np.float32))
    feats = np.asarray(feats, dtype=np.float32).astype(mm_np)
    # [B, F, H, W] -> per sample [p, t, fgrp, fj] = featsT[t*128+p, fgrp*512+fj]
    feats_sh = np.ascontiguousarray(
        feats.reshape(B, FG, FGW, N_T, P).transpose(0, 4, 3, 1, 2)
    )
    # w_proj [E, F] -> wT [p, fc, e] = w_proj.T[fc*128+p, e]
    wT = np.ascontiguousarray(
        np.asarray(w_proj, dtype=np.float32)
        .T.astype(ml_dtypes.bfloat16)
        .reshape(FC, P, E)
        .transpose(1, 0, 2)
    )
    return [
        {
            "outputs_in": outputs_aug[b],
            "feats_in": feats_sh[b],
            "wT_in": wT,
            "bias_in": bias,
        }
        for b in range(B)
    ]


def kernel(outputs, feats, w_proj, b_proj, _trace=False, _trace_kwargs=None,
           _dtype=DTYPE, _build_kwargs=None):
    key = (_dtype, tuple(sorted((_build_kwargs or {}).items())))
    if key not in _CACHE:
        _CACHE[key] = build_module(dtype=_dtype, **(_build_kwargs or {}))
    nc = _CACHE[key]
    in_maps = make_in_maps(outputs, feats, w_proj, b_proj, dtype=_dtype)
    res = run_bass_kernel_spmd(
        nc,
        in_maps,
        core_ids=list(range(N_CORES)),
        trace=_trace,
        **(_trace_kwargs or {}),
    )
    # each core returns out.T [K, E]; transpose back to [E, K] and stack
    out = np.stack([np.asarray(r["out"]).T for r in res.results])
    if _trace:
        _CACHE["last_results"] = res
    return out


# revision 27
# speedup vs baseline: 1.0927x; 1.0927x over previous
"""Trainium2 Bass kernel for nn_Encoder segment-reduce.

Reference computation (per sample b):
    cls = onehot(argmax_k outputs[b])            # [K, HW]
    sizes = cls.sum(HW) + 0.01                   # [K]
    feat_set = feats[b] @ cls.T / sizes          # [F, K]
    out[b] = w_proj @ feat_set + bias            # [E, K]

Kernel strategy (pure data parallel: 1 sample per NeuronCore, 8 cores).

Segment-reduce FIRST (the cheap contraction), projection second:
    feat_setT[k, f] = sum_hw onehot[hw, k] * featsT[hw, f]
with the onehot chunk [128hw, 21] as the PE's stationary operand and featsT
chunks [128hw, 512f] as the moving operand.  The four f-group matmuls of each
hw chunk are packed into the four 32-column groups of the PE array via
tile_position=(0, 32j): the stationary onehot only occupies 21 of 128 array
columns, so the four matmuls execute concurrently (measured 4ns stagger) and
the stream keeps pace with the DMA.  One [128, 512] PSUM tile holds all four
accumulators (f-group j at partitions 32j..32j+21).

feats dtype is fp8 e3m4 (TRN FP8_EXP3): N(0,1) data fits the e3m4 range and
its 4 mantissa bits give rel err ~1.2e-2 end to end (threshold 2e-2), halving
HBM traffic vs bf16: 8.39 MB feats + 1 MB wT (bf16) + 0.34 MB outputs (f32)
~= 9.75 MB/core -- the kernel is DMA-bound at the per-core HBM limit.  The
matmuls run in normal (single-rate) fp8 mode: double-fp8 would upcast
operands to e6m3 and destroy e3m4's 4th mantissa bit (measured 3.0e-2).

All host-side layouts give every DMA >= 2KB contiguous per-partition runs
(wT is pre-permuted to [p, fc, e]; bias rides in the outputs transfer).
DMA order: outputs+bias first (the DVE argmax fills the initial feats
window), feats in 1-2MB blocks, the first quarter of wT (needed by the first
projection round) just before the last feats block, then the rest of wT.
The per-chunk size-count matmul is interleaved with the stream so the PE
never serializes behind the (DVE-paced) argmax; the warm-up burst is sized
to end just as the first feats block lands (a longer burst delays the
stream: the PE executes in order at the cold 1.2 GHz clock).

Tail (all PE stages packed into array tile groups, dummy-matmul fillers in
the dependency gaps so the HAM clock gate keeps the PE at 2.4 GHz):
  - 1/sizes is replicated to all four column groups with one bf16 matmul,
    then fused into the PSUM->SBUF copies (DVE + ACT halves);
  - the 16 [21,128]->[128,21] transposes run 4-at-a-time (one per 32-row
    row group, via a replicated identity) into ONE [128, 16*21] PSUM tile,
    drained by two half-copies (DVE + ACT);
  - the 16 projection matmuls (feat_set chunk stationary, wT moving) run
    4-at-a-time in the four column groups (round r takes f-chunks 4r..4r+3),
    giving four partial [21, 256] sums at partitions 32j; one final matmul
    against the replicated identity combines them, with the bias pre-loaded
    into its PSUM accumulator early via a diagonal-bias matmul.
The output is stored as [K, E]; the host transposes when gathering.

dtype: "fp8" (e3m4 feats, rel err ~1.2e-2) or "bf16" (rel err ~3e-3).
"""

import numpy as np

import concourse.bacc as bacc
import concourse.bass as bass
import concourse.mybir as mybir
import concourse.tile as tile
from concourse.bass import ds, ts
from concourse.bass_utils import run_bass_kernel_spmd
from concourse.masks import make_identity

# Problem shapes (hardcoded per contract)
B = 8
K = 21
H = 64
W = 64
HW = H * W            # 4096
F = 2048
E = 256
P = 128
FC = F // P           # 16 f-chunks of 128
FG = 4                # f-groups of 512 (PE column groups)
FGW = F // FG         # 512
N_T = HW // P         # 32 hw chunks
OUT_AUG = N_T * K + 2  # outputs row + 2 bias values per partition
N_CORES = 8

F32 = mybir.dt.float32
BF16 = mybir.dt.bfloat16
FP8 = mybir.dt.float8e3   # e3m4: 4 mantissa bits

DTYPE = "fp8"         # "fp8" or "bf16"


def build_module(dtype=DTYPE):
    mm_dt = FP8 if dtype == "fp8" else BF16
    nc = bacc.Bacc("TRN2", target_bir_lowering=False, debug=False)

    # outputs host-transposed to [p, t, k] (pixel-major).
    outputs_d = nc.dram_tensor("outputs_in", [P, N_T, K], F32, kind="ExternalInput")
    # featsT host-permuted to [p, t, fgrp, fj]: featsT[t*128+p, fgrp*512+fj].
    feats_d = nc.dram_tensor(
        "feats_in", [P, N_T, FG, FGW], mm_dt, kind="ExternalInput"
    )
    # wT host-permuted to [p, fc, e] = w_proj.T[fc*128+p, e] (contiguous 8KB
    # per-partition runs -- the naive [F, E] layout DMAs in 512B pieces).
    wT_d = nc.dram_tensor("wT_in", [P, FC, E], BF16, kind="ExternalInput")
    bias_d = nc.dram_tensor("bias_in", [E], F32, kind="ExternalInput")
    # out.T -- the host transposes each sample's [K, E] result when gathering.
    out_d = nc.dram_tensor("out", [K, E], F32, kind="ExternalOutput")

    # feats DMA blocks (start chunk, n chunks).
    blocks_a = [(0, 4), (4, 8), (12, 8), (20, 8)]
    blocks_b = [(28, 4)]

    with tile.TileContext(nc) as tc:
        with (
            tc.tile_pool(name="consts", bufs=1) as consts,
            tc.tile_pool(name="feats", bufs=5) as feats_pool,
            tc.tile_pool(name="small", bufs=4) as small,
            tc.tile_pool(name="outp", bufs=1) as outp,
            tc.tile_pool(name="ps_fs", bufs=1, space="PSUM") as ps_fs,
            tc.tile_pool(name="ps_sz", bufs=1, space="PSUM") as ps_sz,
            tc.tile_pool(name="ps_tr", bufs=1, space="PSUM") as ps_tr,
            tc.tile_pool(name="ps_misc", bufs=1, space="PSUM") as ps_misc,
        ):
            # Bulk DMAs in FIFO order on the sync HWDGE queue.
            outputs_sb = consts.tile([P, N_T, K], F32)
            nc.sync.dma_start(out=outputs_sb, in_=outputs_d.ap())
            bias_sb = consts.tile([1, E], F32)
            nc.sync.dma_start(
                out=bias_sb, in_=bias_d.ap().rearrange("(o e) -> o e", o=1)
            )
            feats_r = feats_d.ap()
            wT_sb = consts.tile([P, FC, E], BF16)
            fgs = {}

            def load_feats(block_list):
                for t0, tb in block_list:
                    fg = feats_pool.tile([P, tb, FG, FGW], mm_dt,
                                         name=f"fg{t0}", tag="fg")
                    nc.sync.dma_start(out=fg, in_=feats_r[:, ds(t0, tb)])
                    fgs[t0] = fg

            load_feats(blocks_a)
            load_feats(blocks_b)
            nc.sync.dma_start(out=wT_sb, in_=wT_d.ap())
            blocks = blocks_a + blocks_b

            # The stream needs only ones_b (size counts); everything else --
            # identity masks, bias prep, the replicated-identity matmul --
            # is emitted AFTER the stream so neither the in-order PE queue
            # nor the in-order DVE queue stalls the argmax or the first
            # stream chunks on constant construction (only needed by the
            # tail ~20us later).
            ones_b = consts.tile([P, 2], mm_dt)
            nc.vector.memset(ones_b, 1.0)

            # Phase 1 (DVE only): onehot = (outT == rowmax) per hw chunk.
            oh_all = consts.tile([P, N_T, K], mm_dt)
            for t in range(N_T):
                rowmax = small.tile([P, 1], F32)
                nc.vector.tensor_reduce(
                    rowmax, outputs_sb[:, t, :], mybir.AxisListType.X,
                    mybir.AluOpType.max,
                )
                nc.vector.tensor_scalar(
                    out=oh_all[:, t, :],
                    in0=outputs_sb[:, t, :],
                    scalar1=rowmax,
                    scalar2=None,
                    op0=mybir.AluOpType.is_equal,
                )

            # Segment-reduce stream.  Per hw chunk: one size-count matmul
            # (onehot.T @ ones -> [21, 2]) plus four f-group matmuls packed
            # into the four PE column groups, accumulating [128, 512] PSUM
            # (f-group j at partitions 32j..32j+21) across all 32 chunks.
            fs_ps = ps_fs.tile([P, FGW], F32)
            szT_ps = ps_sz.tile([2, K], F32, tag="sz")
            for t0, tb in blocks:
                fg = fgs[t0]
                for ti in range(tb):
                    t = t0 + ti
                    oh_t = oh_all[:, t, :]
                    nc.tensor.matmul(
                        szT_ps,
                        lhsT=ones_b,
                        rhs=oh_t,
                        start=(t == 0),
                        stop=(t == N_T - 1),
                    )
                    for j in range(FG):
                        nc.tensor.matmul(
                            fs_ps[ds(32 * j, K), :],
                            lhsT=oh_t,
                            rhs=fg[:, ti, j, :],
                            start=(t == 0),
                            stop=(t == N_T - 1),
                            tile_position=(0, 32 * j),
                        )

            # Tail constants (see note above the stream).
            ident = consts.tile([P, P], F32)
            make_identity(nc, ident)
            rep_sb = consts.tile([K, P], BF16)
            nc.vector.memset(rep_sb, 0.0)
            for j in range(FG):
                nc.vector.tensor_copy(rep_sb[:, ds(32 * j, K)], ident[:K, :K])
            ident21_b = consts.tile([K, K], BF16)
            nc.vector.tensor_copy(ident21_b, ident[:K, :K])
            identrep_ps = ps_tr.tile([P, K], F32, tag="t0")
            nc.tensor.matmul(
                identrep_ps, lhsT=rep_sb, rhs=ident21_b,
                start=True, stop=True,
            )
            ident_rep = consts.tile([P, K], BF16)
            nc.vector.tensor_copy(ident_rep, identrep_ps)
            one1_b = consts.tile([1, 1], BF16)
            nc.vector.memset(one1_b, 1.0)
            ones21r = consts.tile([1, K], BF16)
            nc.vector.memset(ones21r, 1.0)
            bias_b = consts.tile([1, E], BF16)
            nc.vector.tensor_copy(bias_b, bias_sb)
            dummy_act = small.tile([1, 2], F32, tag="da")
            nc.scalar.activation(
                out=dummy_act, in_=bias_sb[:, 0:2],
                func=mybir.ActivationFunctionType.Copy,
            )

            # 1/sizes: computed on the [1, 21] row, transposed to a [21, 1]
            # column with a tiny matmul, then replicated to all four column
            # groups on the PE.  The PE sits idle through this window, which
            # also pays down the HAM utilization budget before the tail.
            sizesT = small.tile([1, K], F32, tag="sizesT")
            nc.vector.tensor_scalar_add(sizesT, szT_ps[0:1, :], 0.01)
            recipT = small.tile([1, K], F32, tag="recipT")
            nc.vector.reciprocal(recipT, sizesT)
            recipT_b = small.tile([1, K], BF16, tag="recipTb")
            nc.vector.tensor_copy(recipT_b, recipT)
            recipc_ps = ps_sz.tile([K, 1], F32, name="recipc", tag="sz")
            nc.tensor.matmul(recipc_ps, lhsT=recipT_b, rhs=one1_b,
                             start=True, stop=True)
            recip_b = small.tile([K, 1], BF16, tag="recipb")
            nc.vector.tensor_copy(recip_b, recipc_ps)
            recip_ps = ps_sz.tile([P, 1], F32, name="recip128", tag="sz")
            nc.tensor.matmul(recip_ps, lhsT=rep_sb, rhs=recip_b,
                             start=True, stop=True)
            recip128 = small.tile([P, 1], F32, tag="r128")
            nc.vector.tensor_copy(recip128, recip_ps)

            # Scale by 1/sizes during the PSUM->SBUF copy (DVE + ACT halves).
            fs_sc = consts.tile([P, FGW], BF16)
            nc.vector.tensor_scalar_mul(
                fs_sc[:, 0 : FGW // 2], fs_ps[:, 0 : FGW // 2], recip128
            )
            nc.scalar.activation(
                out=fs_sc[:, ds(FGW // 2, FGW // 2)],
                in_=fs_ps[:, ds(FGW // 2, FGW // 2)],
                func=mybir.ActivationFunctionType.Copy,
                scale=recip128,
            )

            # Transposes, 4 concurrent per round (one per 32-row row group),
            # all into one PSUM tile, drained by two half-copies.  f-chunk
            # fc = 4j + c lives in column group j at free cols 128c..128c+128
            # of fs_sc.
            # (stride padded to 22 elements so each bf16 PSUM write is
            # 4-byte aligned)
            fsT_sb = consts.tile([P, FC, K], BF16)
            for c in range(4):
                trps = []
                for j in range(FG):
                    fc = 4 * j + c
                    trp = ps_tr.tile([P, K], BF16, name=f"trp{fc}",
                                     tag=f"t{j}")
                    nc.tensor.transpose(
                        trp,
                        fs_sc[ds(32 * j, K), ts(c, P)],
                        ident_rep[ds(32 * j, K), :],
                        tile_position=(32 * j, 0),
                    )
                    trps.append((fc, trp))
                for i, (fc, trp) in enumerate(trps):
                    if i % 2 == 0:
                        nc.vector.tensor_copy(fsT_sb[:, fc, :], trp)
                    else:
                        nc.scalar.activation(
                            out=fsT_sb[:, fc, :], in_=trp,
                            func=mybir.ActivationFunctionType.Copy,
                        )

            # Projection, 4 concurrent per round: round r covers f-chunks
            # 4r..4r+3; column group j accumulates partials at partitions 32j.
            proj_ps = ps_misc.tile([P, E], F32, tag="warm", name="proj")
            for r in range(4):
                for j in range(FG):
                    fc = 4 * r + j
                    nc.tensor.matmul(
                        proj_ps[ds(32 * j, K), :],
                        lhsT=fsT_sb[:, fc, :],
                        rhs=wT_sb[:, fc, :],
                        start=(r == 0),
                        stop=(r == 3),
                        tile_position=(0, 32 * j),
                    )
            proj_sb = consts.tile([P, E], BF16)
            nc.vector.tensor_copy(proj_sb[:, 0 : E // 2], proj_ps[:, 0 : E // 2])
            nc.scalar.activation(
                out=proj_sb[:, ds(E // 2, E // 2)],
                in_=proj_ps[:, ds(E // 2, E // 2)],
                func=mybir.ActivationFunctionType.Copy,
            )

            # Combine the four partials with the bias:
            # final[k, e] = bias[e] + sum_j proj[32j+k, e].
            final_ps = ps_misc.tile([K, E], F32, tag="warm", name="final")
            nc.tensor.matmul(final_ps, lhsT=ones21r, rhs=bias_b,
                             start=True, stop=False, skip_group_check=True)
            nc.tensor.matmul(final_ps, lhsT=ident_rep, rhs=proj_sb,
                             start=False, stop=True, skip_group_check=True)
            out_sb = outp.tile([K, E], F32)
            nc.vector.tensor_copy(out_sb[:, 0 : E // 2], final_ps[:, 0 : E // 2])
            nc.scalar.activation(
                out=out_sb[:, ds(E // 2, E // 2)],
                in_=final_ps[:, ds(E // 2, E // 2)],
                func=mybir.ActivationFunctionType.Copy,
            )
            nc.sync.dma_start(out=out_d.ap(), in_=out_sb)

    nc.compile()
    return nc


_CACHE = {}


def make_in_maps(outputs, feats, w_proj, b_proj, dtype=DTYPE):
    import ml_dtypes

    mm_np = ml_dtypes.float8_e3m4 if dtype == "fp8" else ml_dtypes.bfloat16
    outputs = np.asarray(outputs, dtype=np.float32)
    outputs_aug = np.ascontiguousarray(
        outputs.reshape(B, K, N_T, P).transpose(0, 3, 2, 1)
    )
    bias = np.ascontiguousarray(np.asarray(b_proj, dtype=np.float32))
    feats = np.asarray(feats, dtype=np.float32).astype(mm_np)
    # [B, F, H, W] -> per sample [p, t, fgrp, fj] = featsT[t*128+p, fgrp*512+fj]
    feats_sh = np.ascontiguousarray(
        feats.reshape(B, FG, FGW, N_T, P).transpose(0, 4, 3, 1, 2)
    )
    # w_proj [E, F] -> wT [p, fc, e] = w_proj.T[fc*128+p, e]
    wT = np.ascontiguousarray(
        np.asarray(w_proj, dtype=np.float32)
        .T.astype(ml_dtypes.bfloat16)
        .reshape(FC, P, E)
        .transpose(1, 0, 2)
    )
    return [
        {
            "outputs_in": outputs_aug[b],
            "feats_in": feats_sh[b],
            "wT_in": wT,
            "bias_in": bias,
        }
        for b in range(B)
    ]


def kernel(outputs, feats, w_proj, b_proj, _trace=False, _trace_kwargs=None,
           _dtype=DTYPE, _build_kwargs=None):
    key = (_dtype, tuple(sorted((_build_kwargs or {}).items())))
    if key not in _CACHE:
        _CACHE[key] = build_module(dtype=_dtype, **(_build_kwargs or {}))
    nc = _CACHE[key]
    in_maps = make_in_maps(outputs, feats, w_proj, b_proj, dtype=_dtype)
    res = run_bass_kernel_spmd(
        nc,
        in_maps,
        core_ids=list(range(N_CORES)),
        trace=_trace,
        **(_trace_kwargs or {}),
    )
    # each core returns out.T [K, E]; transpose back to [E, K] and stack
    out = np.stack([np.asarray(r["out"]).T for r in res.results])
    if _trace:
        _CACHE["last_results"] = res
    return out
